# revision 22
# baseline (speedup 1.0000x reference)
"""Trainium2 Bass kernel for DirectConv2D (3x3 VALID, NCHW/OIHW).

Problem: x [32, 256, 56, 56] int32 (values 0..7 after clip),
         weight [256, 256, 3, 3] fp32 (small non-negative ints 0..6)
         -> out [32, 256, 54, 54] fp32.

Strategy:
 - Data-parallel across 8 NeuronCores: 4 images per core, weight replicated.
 - Conv decomposed into 9 shifted matmuls (one per kernel tap) accumulated
   in PSUM; contraction over the 256 input channels.
 - Inputs are tiny non-negative integers, so fp8-e4m3 matmuls are exact
   (products <= 42, fp32 PSUM accumulation). DoubleRow perf mode contracts
   all 256 input channels (2 x 128-partition k-tiles) per matmul.
 - Activations live in SBUF as [128 part, chunk 2, img 4, pix 3140]
   (56*56=3136 pixels + 4 pad so every tile can read a full 504-wide
   window). Output computed in tiles of 9 rows x 56 cols = 504 <= 512
   (one PSUM bank); only the 54 valid cols per row are stored.
"""

import sys

sys.path.insert(0, "/opt/trn_rl_repo")

import ml_dtypes
import numpy as np

N_CORES = 8
IMGS = 4  # images per core
H = W = 56
OH = OW = 54
PIX = H * W  # 3136
PIXP = PIX + 4  # padded so kh=2,kw=2 window of width 504 stays in-bounds
ROWS_PER_TILE = 9
N_TILE = ROWS_PER_TILE * W  # 504 (<= 512 fp32 PSUM bank)
N_ROWTILES = OH // ROWS_PER_TILE  # 6

_PROGRAM_CACHE = {}


def _build_program(mode="fp8dr"):
    import concourse.bacc as bacc
    import concourse.mybir as mybir
    import concourse.tile as tile

    nc = bacc.Bacc(
        "TRN2",
        target_bir_lowering=False,
        debug=False,
        enable_asserts=False,
        num_devices=N_CORES,
    )
    dt8 = mybir.dt.float8e4
    dtb = mybir.dt.bfloat16
    dt_in = dt8 if mode == "fp8dr" else dtb

    x_d = nc.dram_tensor("x_sb", [128, 2, IMGS, PIXP], dt_in, kind="ExternalInput").ap()
    w_d = nc.dram_tensor("w_sb", [128, 2, 9, 2, 128], dt_in, kind="ExternalInput").ap()
    out_d = nc.dram_tensor(
        "out", [IMGS, 256, OH, OW], mybir.dt.float32, kind="ExternalOutput"
    ).ap()

    NT486 = ROWS_PER_TILE * OW  # 486 output pixels per row tile
    X0A_END = 1232  # image-0 leading tile: rows 0..21 (covers row tiles 0,1)
    X0M_BASE, X0M_END = 1008, 2140  # image-0 middle tile (row tiles 2,3)
    X0Z_BASE = 2016  # image-0 trailing tile (row tiles 4,5)

    with tile.TileContext(nc) as tc:
        with (
            tc.tile_pool(name="const", bufs=1) as const_pool,
            tc.tile_pool(name="psum", bufs=8, space="PSUM") as psum_pool,
            tc.tile_pool(name="outs", bufs=3) as out_pool,
        ):
            # PE warm-up on zeroed scratch (no DMA deps): ~8 cold DR matmuls
            # run during the input-load window so HAM un-throttles before the
            # real matmul stream starts.
            w_warm = const_pool.tile([128, 2, 128], dt_in)
            x_warm = const_pool.tile([128, 2, 544], dt_in)
            nc.vector.memset(w_warm, 0.0)
            nc.vector.memset(x_warm, 0.0)
            pt_warm = psum_pool.tile([128, NT486], mybir.dt.float32, tag="pt")
            N_WARM = 10
            for i in range(N_WARM):
                rhs_w = x_warm[:, :, 0:N_TILE].rearrange(
                    "p c (r q) -> p c r q", q=W
                )[:, :, :, 0:OW]
                if mode == "fp8dr":
                    nc.tensor.matmul(
                        pt_warm, w_warm, rhs_w,
                        start=(i == 0), stop=(i == N_WARM - 1),
                        perf_mode=mybir.MatmulPerfMode.DoubleRow,
                    )
                else:
                    nc.tensor.matmul(
                        pt_warm, w_warm[:, 0], rhs_w[:, 0],
                        start=(i == 0), stop=(i == N_WARM - 1),
                    )

            wt = const_pool.tile([128, 2, 9, 2, 128], dt_in)
            # Per-image x tiles so matmul deps only cover the image they read
            # (dep tracking is per-tile). dma_start issue serializes ~0.6us
            # per engine sequencer and each ring FIFOs its transfers, so the
            # bytes gating the first matmul group (leading rows of image 0 +
            # first oc0 weight tap) go at the head of the sync ring; the rest
            # is ordered by first-use time across both rings.
            xt0a = const_pool.tile([128, 2, X0A_END], dt_in)
            xt0m = const_pool.tile([128, 2, X0M_END - X0M_BASE], dt_in)
            xt0z = const_pool.tile([128, 2, PIXP - X0Z_BASE], dt_in)
            xts = [None] + [
                const_pool.tile([128, 2, PIXP], dt_in, name=f"xt{n}", tag=f"xt{n}")
                for n in (1, 2, 3)
            ]
            # The first matmul group needs xt0a's leading chunks + the first
            # weight tap: split them across both rings so both completion
            # sems fire as early as possible. Everything else is ordered by
            # first-use time.
            # sync ring: image-0 lead (c0), weights, image 1
            nc.sync.dma_start(out=xt0a[:, 0, 0:620], in_=x_d[:, 0, 0, 0:620])
            nc.sync.dma_start(out=wt[:, 0, 0], in_=w_d[:, 0, 0])
            nc.sync.dma_start(out=wt[:, 0, 1:], in_=w_d[:, 0, 1:])
            nc.sync.dma_start(out=wt[:, 1], in_=w_d[:, 1])
            for c in range(2):
                nc.sync.dma_start(out=xts[1][:, c], in_=x_d[:, c, 1])
            # scalar ring: image-0 lead (c1), rest of image 0, images 2-3
            nc.scalar.dma_start(out=xt0a[:, 1, 0:620], in_=x_d[:, 1, 0, 0:620])
            for c in range(2):
                nc.scalar.dma_start(
                    out=xt0a[:, c, 620:], in_=x_d[:, c, 0, 620:X0A_END]
                )
            for c in range(2):
                nc.scalar.dma_start(
                    out=xt0m[:, c], in_=x_d[:, c, 0, X0M_BASE:X0M_END]
                )
            for c in range(2):
                nc.scalar.dma_start(out=xt0z[:, c], in_=x_d[:, c, 0, X0Z_BASE:])
            for n in (2, 3):
                for c in range(2):
                    nc.scalar.dma_start(out=xts[n][:, c], in_=x_d[:, c, n])

            def x_src(n, t):
                """(x tile, pixel base) holding rows needed by row tile t."""
                if n == 0:
                    if t < 2:
                        return xt0a, 0
                    if t < 4:
                        return xt0m, X0M_BASE
                    return xt0z, X0Z_BASE
                return xts[n], 0

            for n in range(IMGS):
                for oc in range(2):
                    # staging for a full (n, oc) output block: dense 54x54
                    # rows so stores move 11.7KB-contiguous lines/partition.
                    ot = out_pool.tile([128, OH * OW], mybir.dt.float32)
                    for t in range(N_ROWTILES):
                        h0 = t * ROWS_PER_TILE
                        xsrc, xbase = x_src(n, t)
                        pt = psum_pool.tile([128, NT486], mybir.dt.float32)
                        k = 0
                        for kh in range(3):
                            for kw in range(3):
                                off = (h0 + kh) * W + kw - xbase
                                # strided moving AP skips the 2 junk cols per
                                # row: [128, 2, 9 rows (stride 56), 54 cols]
                                if mode == "fp8dr":
                                    rhs = xsrc[:, :, off : off + N_TILE].rearrange(
                                        "p c (r q) -> p c r q", q=W
                                    )[:, :, :, 0:OW]
                                    nc.tensor.matmul(
                                        pt,
                                        wt[:, oc, k, :, :],
                                        rhs,
                                        start=(k == 0),
                                        stop=(k == 8),
                                        perf_mode=mybir.MatmulPerfMode.DoubleRow,
                                    )
                                else:
                                    for c in range(2):
                                        rhs = xsrc[:, c, off : off + N_TILE].rearrange(
                                            "p (r q) -> p r q", q=W
                                        )[:, :, 0:OW]
                                        nc.tensor.matmul(
                                            pt,
                                            wt[:, oc, k, c, :],
                                            rhs,
                                            start=(k == 0 and c == 0),
                                            stop=(k == 8 and c == 1),
                                        )
                                k += 1
                        nc.vector.tensor_copy(
                            out=ot[:, t * NT486 : (t + 1) * NT486], in_=pt
                        )
                        last_block = n == IMGS - 1 and oc == 1
                        if last_block:
                            # fine-grained stores on the final block: pairs
                            # early, singles at the end so the final store
                            # (and its completion latency) is small.
                            if t in (1, 3):
                                nc.sync.dma_start(
                                    out=out_d[n, oc * 128 : (oc + 1) * 128,
                                              h0 - ROWS_PER_TILE : h0 + ROWS_PER_TILE, :],
                                    in_=ot[:, (t - 1) * NT486 : (t + 1) * NT486].rearrange(
                                        "p (h w) -> p h w", w=OW
                                    ),
                                )
                            elif t >= 4:
                                # t=4 on sync, t=5 on scalar: the two final
                                # stores' start/completion latencies overlap.
                                eng = nc.sync if t == 4 else nc.scalar
                                eng.dma_start(
                                    out=out_d[n, oc * 128 : (oc + 1) * 128,
                                              h0 : h0 + ROWS_PER_TILE, :],
                                    in_=ot[:, t * NT486 : (t + 1) * NT486].rearrange(
                                        "p (h w) -> p h w", w=OW
                                    ),
                                )
                    if not last_block:
                        nc.sync.dma_start(
                            out=out_d[n, oc * 128 : (oc + 1) * 128, :, :],
                            in_=ot.rearrange("p (h w) -> p h w", w=OW),
                        )
    nc.compile()
    return nc


def get_program(mode="fp8dr"):
    if mode not in _PROGRAM_CACHE:
        _PROGRAM_CACHE[mode] = _build_program(mode)
    return _PROGRAM_CACHE[mode]


def _np_dtype(mode):
    return ml_dtypes.float8_e4m3 if mode == "fp8dr" else ml_dtypes.bfloat16


def prep_weight(weight, mode="fp8dr"):
    """weight [256, 256, 3, 3] OIHW fp32 -> w_sb [128 ki, 2 oc, 9 tap, 2 c, 128 m]."""
    wq = weight.astype(np.int32).astype(np.float32)
    wq = wq.reshape(2, 128, 2, 128, 3, 3)  # [oc, m, c, ki, kh, kw]
    w_sb = np.ascontiguousarray(wq.transpose(3, 0, 4, 5, 2, 1))  # [ki, oc, kh, kw, c, m]
    w_sb = w_sb.reshape(128, 2, 9, 2, 128)
    return w_sb.astype(_np_dtype(mode))


def prep_x_core(x_core, mode="fp8dr"):
    """x_core [IMGS, 256, 56, 56] int32 -> x_sb [128 ki, 2 c, IMGS, PIXP]."""
    xq = np.clip(x_core.astype(np.int32), 0, 7).astype(np.float32)
    xq = xq.reshape(IMGS, 2, 128, PIX)  # [n, c, ki, pix]
    x_sb = np.zeros((128, 2, IMGS, PIXP), np.float32)
    x_sb[:, :, :, :PIX] = xq.transpose(2, 1, 0, 3)
    return x_sb.astype(_np_dtype(mode))


def make_in_maps(x, weight, mode="fp8dr"):
    w_sb = prep_weight(weight, mode)
    return [
        {"x_sb": prep_x_core(x[c * IMGS : (c + 1) * IMGS], mode), "w_sb": w_sb}
        for c in range(N_CORES)
    ]


def kernel(x, weight):
    import time

    from concourse.bass_utils import run_bass_kernel_spmd

    mode = "fp8dr"
    nc = get_program(mode)
    in_maps = make_in_maps(np.asarray(x), np.asarray(weight), mode)
    last_err = None
    for attempt in range(3):
        try:
            res = run_bass_kernel_spmd(nc, in_maps, list(range(N_CORES)))
            break
        except Exception as e:  # transient NRT_EXEC_UNIT_UNRECOVERABLE flakes
            last_err = e
            time.sleep(2.0)
    else:
        raise last_err
    return np.concatenate(
        [res.results[c]["out"] for c in range(N_CORES)], axis=0
    ).astype(np.float32)


# revision 23
# speedup vs baseline: 1.0019x; 1.0019x over previous
"""Trainium2 Bass kernel for DirectConv2D (3x3 VALID, NCHW/OIHW).

Problem: x [32, 256, 56, 56] int32 (values 0..7 after clip),
         weight [256, 256, 3, 3] fp32 (small non-negative ints 0..6)
         -> out [32, 256, 54, 54] fp32.

Strategy:
 - Data-parallel across 8 NeuronCores: 4 images per core, weight replicated.
 - Conv decomposed into 9 shifted matmuls (one per kernel tap) accumulated
   in PSUM; contraction over the 256 input channels.
 - Inputs are tiny non-negative integers, so fp8-e4m3 matmuls are exact
   (products <= 42, fp32 PSUM accumulation). DoubleRow perf mode contracts
   all 256 input channels (2 x 128-partition k-tiles) per matmul.
 - Activations live in SBUF as [128 part, chunk 2, img 4, pix 3140]
   (56*56=3136 pixels + 4 pad so every tile can read a full 504-wide
   window). Output computed in tiles of 9 rows x 56 cols = 504 <= 512
   (one PSUM bank); only the 54 valid cols per row are stored.
"""

import sys

sys.path.insert(0, "/opt/trn_rl_repo")

import ml_dtypes
import numpy as np

N_CORES = 8
IMGS = 4  # images per core
H = W = 56
OH = OW = 54
PIX = H * W  # 3136
PIXP = PIX + 4  # padded so kh=2,kw=2 window of width 504 stays in-bounds
ROWS_PER_TILE = 9
N_TILE = ROWS_PER_TILE * W  # 504 (<= 512 fp32 PSUM bank)
N_ROWTILES = OH // ROWS_PER_TILE  # 6

_PROGRAM_CACHE = {}


def _build_program(mode="fp8dr"):
    import concourse.bacc as bacc
    import concourse.mybir as mybir
    import concourse.tile as tile

    nc = bacc.Bacc(
        "TRN2",
        target_bir_lowering=False,
        debug=False,
        enable_asserts=False,
        num_devices=N_CORES,
    )
    dt8 = mybir.dt.float8e4
    dtb = mybir.dt.bfloat16
    dt_in = dt8 if mode == "fp8dr" else dtb

    x_d = nc.dram_tensor("x_sb", [128, 2, IMGS, PIXP], dt_in, kind="ExternalInput").ap()
    w_d = nc.dram_tensor("w_sb", [128, 2, 9, 2, 128], dt_in, kind="ExternalInput").ap()
    out_d = nc.dram_tensor(
        "out", [IMGS, 256, OH, OW], mybir.dt.float32, kind="ExternalOutput"
    ).ap()

    NT486 = ROWS_PER_TILE * OW  # 486 output pixels per row tile
    X0A_END = 1232  # image-0 leading tile: rows 0..21 (covers row tiles 0,1)
    X0M_BASE, X0M_END = 1008, 2140  # image-0 middle tile (row tiles 2,3)
    X0Z_BASE = 2016  # image-0 trailing tile (row tiles 4,5)

    with tile.TileContext(nc) as tc:
        with (
            tc.tile_pool(name="const", bufs=1) as const_pool,
            tc.tile_pool(name="psum", bufs=8, space="PSUM") as psum_pool,
            tc.tile_pool(name="outs", bufs=3) as out_pool,
        ):
            # PE warm-up on zeroed scratch (no DMA deps): ~8 cold DR matmuls
            # run during the input-load window so HAM un-throttles before the
            # real matmul stream starts.
            w_warm = const_pool.tile([128, 2, 128], dt_in)
            x_warm = const_pool.tile([128, 2, 544], dt_in)
            nc.gpsimd.memset(w_warm, 0.0)
            nc.gpsimd.memset(x_warm, 0.0)
            pt_warm = psum_pool.tile([128, NT486], mybir.dt.float32, tag="pt")
            N_WARM = 10
            for i in range(N_WARM):
                rhs_w = x_warm[:, :, 0:N_TILE].rearrange(
                    "p c (r q) -> p c r q", q=W
                )[:, :, :, 0:OW]
                if mode == "fp8dr":
                    nc.tensor.matmul(
                        pt_warm, w_warm, rhs_w,
                        start=(i == 0), stop=(i == N_WARM - 1),
                        perf_mode=mybir.MatmulPerfMode.DoubleRow,
                    )
                else:
                    nc.tensor.matmul(
                        pt_warm, w_warm[:, 0], rhs_w[:, 0],
                        start=(i == 0), stop=(i == N_WARM - 1),
                    )

            wt = const_pool.tile([128, 2, 9, 2, 128], dt_in)
            # Per-image x tiles so matmul deps only cover the image they read
            # (dep tracking is per-tile). dma_start issue serializes ~0.6us
            # per engine sequencer and each ring FIFOs its transfers, so the
            # bytes gating the first matmul group (leading rows of image 0 +
            # first oc0 weight tap) go at the head of the sync ring; the rest
            # is ordered by first-use time across both rings.
            xt0a = const_pool.tile([128, 2, X0A_END], dt_in)
            xt0m = const_pool.tile([128, 2, X0M_END - X0M_BASE], dt_in)
            xt0z = const_pool.tile([128, 2, PIXP - X0Z_BASE], dt_in)
            xts = [None] + [
                const_pool.tile([128, 2, PIXP], dt_in, name=f"xt{n}", tag=f"xt{n}")
                for n in (1, 2, 3)
            ]
            # The first matmul group needs xt0a's leading chunks + the first
            # weight tap: split them across both rings so both completion
            # sems fire as early as possible. Everything else is ordered by
            # first-use time.
            # sync ring: image-0 lead (c0), weights, image 1
            nc.sync.dma_start(out=xt0a[:, 0, 0:620], in_=x_d[:, 0, 0, 0:620])
            nc.sync.dma_start(out=wt[:, 0, 0], in_=w_d[:, 0, 0])
            nc.sync.dma_start(out=wt[:, 0, 1:], in_=w_d[:, 0, 1:])
            nc.sync.dma_start(out=wt[:, 1], in_=w_d[:, 1])
            for c in range(2):
                nc.sync.dma_start(out=xts[1][:, c], in_=x_d[:, c, 1])
            # scalar ring: image-0 lead (c1), rest of image 0, images 2-3
            nc.scalar.dma_start(out=xt0a[:, 1, 0:620], in_=x_d[:, 1, 0, 0:620])
            for c in range(2):
                nc.scalar.dma_start(
                    out=xt0a[:, c, 620:], in_=x_d[:, c, 0, 620:X0A_END]
                )
            for c in range(2):
                nc.scalar.dma_start(
                    out=xt0m[:, c], in_=x_d[:, c, 0, X0M_BASE:X0M_END]
                )
            for c in range(2):
                nc.scalar.dma_start(out=xt0z[:, c], in_=x_d[:, c, 0, X0Z_BASE:])
            for n in (2, 3):
                for c in range(2):
                    nc.scalar.dma_start(out=xts[n][:, c], in_=x_d[:, c, n])

            def x_src(n, t):
                """(x tile, pixel base) holding rows needed by row tile t."""
                if n == 0:
                    if t < 2:
                        return xt0a, 0
                    if t < 4:
                        return xt0m, X0M_BASE
                    return xt0z, X0Z_BASE
                return xts[n], 0

            for n in range(IMGS):
                for oc in range(2):
                    # staging for a full (n, oc) output block: dense 54x54
                    # rows so stores move 11.7KB-contiguous lines/partition.
                    ot = out_pool.tile([128, OH * OW], mybir.dt.float32)
                    for t in range(N_ROWTILES):
                        h0 = t * ROWS_PER_TILE
                        xsrc, xbase = x_src(n, t)
                        pt = psum_pool.tile([128, NT486], mybir.dt.float32)
                        k = 0
                        for kh in range(3):
                            for kw in range(3):
                                off = (h0 + kh) * W + kw - xbase
                                # strided moving AP skips the 2 junk cols per
                                # row: [128, 2, 9 rows (stride 56), 54 cols]
                                if mode == "fp8dr":
                                    rhs = xsrc[:, :, off : off + N_TILE].rearrange(
                                        "p c (r q) -> p c r q", q=W
                                    )[:, :, :, 0:OW]
                                    nc.tensor.matmul(
                                        pt,
                                        wt[:, oc, k, :, :],
                                        rhs,
                                        start=(k == 0),
                                        stop=(k == 8),
                                        perf_mode=mybir.MatmulPerfMode.DoubleRow,
                                    )
                                else:
                                    for c in range(2):
                                        rhs = xsrc[:, c, off : off + N_TILE].rearrange(
                                            "p (r q) -> p r q", q=W
                                        )[:, :, 0:OW]
                                        nc.tensor.matmul(
                                            pt,
                                            wt[:, oc, k, c, :],
                                            rhs,
                                            start=(k == 0 and c == 0),
                                            stop=(k == 8 and c == 1),
                                        )
                                k += 1
                        nc.vector.tensor_copy(
                            out=ot[:, t * NT486 : (t + 1) * NT486], in_=pt
                        )
                        last_block = n == IMGS - 1 and oc == 1
                        if last_block:
                            # fine-grained stores on the final block: pairs
                            # early, singles at the end so the final store
                            # (and its completion latency) is small.
                            if t in (1, 3):
                                nc.sync.dma_start(
                                    out=out_d[n, oc * 128 : (oc + 1) * 128,
                                              h0 - ROWS_PER_TILE : h0 + ROWS_PER_TILE, :],
                                    in_=ot[:, (t - 1) * NT486 : (t + 1) * NT486].rearrange(
                                        "p (h w) -> p h w", w=OW
                                    ),
                                )
                            elif t >= 4:
                                # t=4 on sync, t=5 on scalar: the two final
                                # stores' start/completion latencies overlap.
                                eng = nc.sync if t == 4 else nc.scalar
                                eng.dma_start(
                                    out=out_d[n, oc * 128 : (oc + 1) * 128,
                                              h0 : h0 + ROWS_PER_TILE, :],
                                    in_=ot[:, t * NT486 : (t + 1) * NT486].rearrange(
                                        "p (h w) -> p h w", w=OW
                                    ),
                                )
                    if not last_block:
                        nc.sync.dma_start(
                            out=out_d[n, oc * 128 : (oc + 1) * 128, :, :],
                            in_=ot.rearrange("p (h w) -> p h w", w=OW),
                        )
    nc.compile()
    return nc


def get_program(mode="fp8dr"):
    if mode not in _PROGRAM_CACHE:
        _PROGRAM_CACHE[mode] = _build_program(mode)
    return _PROGRAM_CACHE[mode]


def _np_dtype(mode):
    return ml_dtypes.float8_e4m3 if mode == "fp8dr" else ml_dtypes.bfloat16


def prep_weight(weight, mode="fp8dr"):
    """weight [256, 256, 3, 3] OIHW fp32 -> w_sb [128 ki, 2 oc, 9 tap, 2 c, 128 m]."""
    wq = weight.astype(np.int32).astype(np.float32)
    wq = wq.reshape(2, 128, 2, 128, 3, 3)  # [oc, m, c, ki, kh, kw]
    w_sb = np.ascontiguousarray(wq.transpose(3, 0, 4, 5, 2, 1))  # [ki, oc, kh, kw, c, m]
    w_sb = w_sb.reshape(128, 2, 9, 2, 128)
    return w_sb.astype(_np_dtype(mode))


def prep_x_core(x_core, mode="fp8dr"):
    """x_core [IMGS, 256, 56, 56] int32 -> x_sb [128 ki, 2 c, IMGS, PIXP]."""
    xq = np.clip(x_core.astype(np.int32), 0, 7).astype(np.float32)
    xq = xq.reshape(IMGS, 2, 128, PIX)  # [n, c, ki, pix]
    x_sb = np.zeros((128, 2, IMGS, PIXP), np.float32)
    x_sb[:, :, :, :PIX] = xq.transpose(2, 1, 0, 3)
    return x_sb.astype(_np_dtype(mode))


def make_in_maps(x, weight, mode="fp8dr"):
    w_sb = prep_weight(weight, mode)
    return [
        {"x_sb": prep_x_core(x[c * IMGS : (c + 1) * IMGS], mode), "w_sb": w_sb}
        for c in range(N_CORES)
    ]


def kernel(x, weight):
    import time

    from concourse.bass_utils import run_bass_kernel_spmd

    mode = "fp8dr"
    nc = get_program(mode)
    in_maps = make_in_maps(np.asarray(x), np.asarray(weight), mode)
    last_err = None
    for attempt in range(3):
        try:
            res = run_bass_kernel_spmd(nc, in_maps, list(range(N_CORES)))
            break
        except Exception as e:  # transient NRT_EXEC_UNIT_UNRECOVERABLE flakes
            last_err = e
            time.sleep(2.0)
    else:
        raise last_err
    return np.concatenate(
        [res.results[c]["out"] for c in range(N_CORES)], axis=0
    ).astype(np.float32)


# revision 24
# speedup vs baseline: 1.0086x; 1.0067x over previous
"""Trainium2 Bass kernel for DirectConv2D (3x3 VALID, NCHW/OIHW).

Problem: x [32, 256, 56, 56] int32 (values 0..7 after clip),
         weight [256, 256, 3, 3] fp32 (small non-negative ints 0..6)
         -> out [32, 256, 54, 54] fp32.

Strategy:
 - Data-parallel across 8 NeuronCores: 4 images per core, weight replicated.
 - Conv decomposed into 9 shifted matmuls (one per kernel tap) accumulated
   in PSUM; contraction over the 256 input channels.
 - Inputs are tiny non-negative integers, so fp8-e4m3 matmuls are exact
   (products <= 42, fp32 PSUM accumulation). DoubleRow perf mode contracts
   all 256 input channels (2 x 128-partition k-tiles) per matmul.
 - Activations live in SBUF as [128 part, chunk 2, img 4, pix 3140]
   (56*56=3136 pixels + 4 pad so every tile can read a full 504-wide
   window). Output computed in tiles of 9 rows x 56 cols = 504 <= 512
   (one PSUM bank); only the 54 valid cols per row are stored.
"""

import sys

sys.path.insert(0, "/opt/trn_rl_repo")

import ml_dtypes
import numpy as np

N_CORES = 8
IMGS = 4  # images per core
H = W = 56
OH = OW = 54
PIX = H * W  # 3136
PIXP = PIX + 4  # padded so kh=2,kw=2 window of width 504 stays in-bounds
ROWS_PER_TILE = 9
N_TILE = ROWS_PER_TILE * W  # 504 (<= 512 fp32 PSUM bank)
N_ROWTILES = OH // ROWS_PER_TILE  # 6

_PROGRAM_CACHE = {}


def _build_program(mode="fp8dr"):
    import concourse.bacc as bacc
    import concourse.mybir as mybir
    import concourse.tile as tile

    nc = bacc.Bacc(
        "TRN2",
        target_bir_lowering=False,
        debug=False,
        enable_asserts=False,
        num_devices=N_CORES,
    )
    dt8 = mybir.dt.float8e4
    dtb = mybir.dt.bfloat16
    dt_in = dt8 if mode == "fp8dr" else dtb

    x_d = nc.dram_tensor("x_sb", [128, 2, IMGS, PIXP], dt_in, kind="ExternalInput").ap()
    w_d = nc.dram_tensor("w_sb", [128, 2, 9, 2, 128], dt_in, kind="ExternalInput").ap()
    out_d = nc.dram_tensor(
        "out", [IMGS, 256, OH, OW], mybir.dt.float32, kind="ExternalOutput"
    ).ap()

    NT486 = ROWS_PER_TILE * OW  # 486 output pixels per row tile
    X0A_END = 1232  # image-0 leading tile: rows 0..21 (covers row tiles 0,1)
    X0M_BASE, X0M_END = 1008, 2140  # image-0 middle tile (row tiles 2,3)
    X0Z_BASE = 2016  # image-0 trailing tile (row tiles 4,5)

    with tile.TileContext(nc) as tc:
        with (
            tc.tile_pool(name="const", bufs=1) as const_pool,
            tc.tile_pool(name="psum", bufs=8, space="PSUM") as psum_pool,
            tc.tile_pool(name="outs", bufs=3) as out_pool,
        ):
            # PE warm-up on zeroed scratch (no DMA deps): ~8 cold DR matmuls
            # run during the input-load window so HAM un-throttles before the
            # real matmul stream starts.
            w_warm = const_pool.tile([128, 2, 128], dt_in)
            x_warm = const_pool.tile([128, 2, 544], dt_in)
            nc.gpsimd.memset(w_warm, 0.0)
            nc.gpsimd.memset(x_warm, 0.0)
            pt_warm = psum_pool.tile([128, NT486], mybir.dt.float32, tag="pt")
            N_WARM = 10
            for i in range(N_WARM):
                rhs_w = x_warm[:, :, 0:N_TILE].rearrange(
                    "p c (r q) -> p c r q", q=W
                )[:, :, :, 0:OW]
                if mode == "fp8dr":
                    nc.tensor.matmul(
                        pt_warm, w_warm, rhs_w,
                        start=(i == 0), stop=(i == N_WARM - 1),
                        perf_mode=mybir.MatmulPerfMode.DoubleRow,
                    )
                else:
                    nc.tensor.matmul(
                        pt_warm, w_warm[:, 0], rhs_w[:, 0],
                        start=(i == 0), stop=(i == N_WARM - 1),
                    )

            wt = const_pool.tile([128, 2, 9, 2, 128], dt_in)
            # Per-image x tiles so matmul deps only cover the image they read
            # (dep tracking is per-tile). dma_start issue serializes ~0.6us
            # per engine sequencer and each ring FIFOs its transfers, so the
            # bytes gating the first matmul group (leading rows of image 0 +
            # first oc0 weight tap) go at the head of the sync ring; the rest
            # is ordered by first-use time across both rings.
            xt0a = const_pool.tile([128, 2, X0A_END], dt_in)
            xt0m = const_pool.tile([128, 2, X0M_END - X0M_BASE], dt_in)
            xt0z = const_pool.tile([128, 2, PIXP - X0Z_BASE], dt_in)
            xts = [None] + [
                const_pool.tile([128, 2, PIXP], dt_in, name=f"xt{n}", tag=f"xt{n}")
                for n in (1, 2, 3)
            ]
            # The first matmul group needs xt0a's leading chunks + the first
            # weight tap: split them across both rings so both completion
            # sems fire as early as possible. Everything else is ordered by
            # first-use time.
            # sync ring: image-0 lead (c0), weights, image 1
            nc.sync.dma_start(out=wt[:, 0, 0], in_=w_d[:, 0, 0])
            nc.sync.dma_start(out=xt0a[:, 0, 0:620], in_=x_d[:, 0, 0, 0:620])
            nc.sync.dma_start(out=wt[:, 0, 1:], in_=w_d[:, 0, 1:])
            nc.sync.dma_start(out=wt[:, 1], in_=w_d[:, 1])
            for c in range(2):
                nc.sync.dma_start(out=xts[1][:, c], in_=x_d[:, c, 1])
            # scalar ring: image-0 lead (c1), rest of image 0, images 2-3
            nc.scalar.dma_start(out=xt0a[:, 1, 0:620], in_=x_d[:, 1, 0, 0:620])
            for c in range(2):
                nc.scalar.dma_start(
                    out=xt0a[:, c, 620:], in_=x_d[:, c, 0, 620:X0A_END]
                )
            for c in range(2):
                nc.scalar.dma_start(
                    out=xt0m[:, c], in_=x_d[:, c, 0, X0M_BASE:X0M_END]
                )
            for c in range(2):
                nc.scalar.dma_start(out=xt0z[:, c], in_=x_d[:, c, 0, X0Z_BASE:])
            for n in (2, 3):
                for c in range(2):
                    nc.scalar.dma_start(out=xts[n][:, c], in_=x_d[:, c, n])

            def x_src(n, t):
                """(x tile, pixel base) holding rows needed by row tile t."""
                if n == 0:
                    if t < 2:
                        return xt0a, 0
                    if t < 4:
                        return xt0m, X0M_BASE
                    return xt0z, X0Z_BASE
                return xts[n], 0

            for n in range(IMGS):
                for oc in range(2):
                    # staging for a full (n, oc) output block: dense 54x54
                    # rows so stores move 11.7KB-contiguous lines/partition.
                    ot = out_pool.tile([128, OH * OW], mybir.dt.float32)
                    for t in range(N_ROWTILES):
                        h0 = t * ROWS_PER_TILE
                        xsrc, xbase = x_src(n, t)
                        pt = psum_pool.tile([128, NT486], mybir.dt.float32)
                        k = 0
                        for kh in range(3):
                            for kw in range(3):
                                off = (h0 + kh) * W + kw - xbase
                                # strided moving AP skips the 2 junk cols per
                                # row: [128, 2, 9 rows (stride 56), 54 cols]
                                if mode == "fp8dr":
                                    rhs = xsrc[:, :, off : off + N_TILE].rearrange(
                                        "p c (r q) -> p c r q", q=W
                                    )[:, :, :, 0:OW]
                                    nc.tensor.matmul(
                                        pt,
                                        wt[:, oc, k, :, :],
                                        rhs,
                                        start=(k == 0),
                                        stop=(k == 8),
                                        perf_mode=mybir.MatmulPerfMode.DoubleRow,
                                    )
                                else:
                                    for c in range(2):
                                        rhs = xsrc[:, c, off : off + N_TILE].rearrange(
                                            "p (r q) -> p r q", q=W
                                        )[:, :, 0:OW]
                                        nc.tensor.matmul(
                                            pt,
                                            wt[:, oc, k, c, :],
                                            rhs,
                                            start=(k == 0 and c == 0),
                                            stop=(k == 8 and c == 1),
                                        )
                                k += 1
                        nc.vector.tensor_copy(
                            out=ot[:, t * NT486 : (t + 1) * NT486], in_=pt
                        )
                        last_block = n == IMGS - 1 and oc == 1
                        if last_block:
                            # fine-grained stores on the final block: pairs
                            # early, singles at the end so the final store
                            # (and its completion latency) is small.
                            if t in (1, 3):
                                nc.sync.dma_start(
                                    out=out_d[n, oc * 128 : (oc + 1) * 128,
                                              h0 - ROWS_PER_TILE : h0 + ROWS_PER_TILE, :],
                                    in_=ot[:, (t - 1) * NT486 : (t + 1) * NT486].rearrange(
                                        "p (h w) -> p h w", w=OW
                                    ),
                                )
                            elif t >= 4:
                                # t=4 on sync, t=5 on scalar: the two final
                                # stores' start/completion latencies overlap.
                                eng = nc.sync if t == 4 else nc.scalar
                                eng.dma_start(
                                    out=out_d[n, oc * 128 : (oc + 1) * 128,
                                              h0 : h0 + ROWS_PER_TILE, :],
                                    in_=ot[:, t * NT486 : (t + 1) * NT486].rearrange(
                                        "p (h w) -> p h w", w=OW
                                    ),
                                )
                    if not last_block:
                        nc.sync.dma_start(
                            out=out_d[n, oc * 128 : (oc + 1) * 128, :, :],
                            in_=ot.rearrange("p (h w) -> p h w", w=OW),
                        )
    nc.compile()
    return nc


def get_program(mode="fp8dr"):
    if mode not in _PROGRAM_CACHE:
        _PROGRAM_CACHE[mode] = _build_program(mode)
    return _PROGRAM_CACHE[mode]


def _np_dtype(mode):
    return ml_dtypes.float8_e4m3 if mode == "fp8dr" else ml_dtypes.bfloat16


def prep_weight(weight, mode="fp8dr"):
    """weight [256, 256, 3, 3] OIHW fp32 -> w_sb [128 ki, 2 oc, 9 tap, 2 c, 128 m]."""
    wq = weight.astype(np.int32).astype(np.float32)
    wq = wq.reshape(2, 128, 2, 128, 3, 3)  # [oc, m, c, ki, kh, kw]
    w_sb = np.ascontiguousarray(wq.transpose(3, 0, 4, 5, 2, 1))  # [ki, oc, kh, kw, c, m]
    w_sb = w_sb.reshape(128, 2, 9, 2, 128)
    return w_sb.astype(_np_dtype(mode))


def prep_x_core(x_core, mode="fp8dr"):
    """x_core [IMGS, 256, 56, 56] int32 -> x_sb [128 ki, 2 c, IMGS, PIXP]."""
    xq = np.clip(x_core.astype(np.int32), 0, 7).astype(np.float32)
    xq = xq.reshape(IMGS, 2, 128, PIX)  # [n, c, ki, pix]
    x_sb = np.zeros((128, 2, IMGS, PIXP), np.float32)
    x_sb[:, :, :, :PIX] = xq.transpose(2, 1, 0, 3)
    return x_sb.astype(_np_dtype(mode))


def make_in_maps(x, weight, mode="fp8dr"):
    w_sb = prep_weight(weight, mode)
    return [
        {"x_sb": prep_x_core(x[c * IMGS : (c + 1) * IMGS], mode), "w_sb": w_sb}
        for c in range(N_CORES)
    ]


def kernel(x, weight):
    import time

    from concourse.bass_utils import run_bass_kernel_spmd

    mode = "fp8dr"
    nc = get_program(mode)
    in_maps = make_in_maps(np.asarray(x), np.asarray(weight), mode)
    last_err = None
    for attempt in range(3):
        try:
            res = run_bass_kernel_spmd(nc, in_maps, list(range(N_CORES)))
            break
        except Exception as e:  # transient NRT_EXEC_UNIT_UNRECOVERABLE flakes
            last_err = e
            time.sleep(2.0)
    else:
        raise last_err
    return np.concatenate(
        [res.results[c]["out"] for c in range(N_CORES)], axis=0
    ).astype(np.float32)
